# revision 3
# baseline (speedup 1.0000x reference)
"""nn_Encoder_Decoder kernel for 8 trn2 NeuronCores (Bass/Tile, SPMD).

Contract: kernel(**inputs) takes the FULL unsharded inputs of
reference.setup_inputs() and returns the FULL output (loss scalar, at[512,512]).

Sharding strategy (per spec hint, adapted):
  - The 256-step LSTM scan is replicated on all 8 cores: it is latency-bound
    (per-step time is independent of batch), and the batch-contracted
    attention einsum needs full-batch hs on every core anyway.
  - Per-core shards: 1/8 of the attention output columns (one-hot-selected
    e-slice of hs kept during the scan) and 1/8 of the vocab for the
    output-projection softmax denominator (tensor parallel over [V,H]).
  - Input-side gate projections (x_t @ W_ih.T + b) are precomputed as one big
    bf16 matmul over all 16384 tokens, staged in DRAM, streamed into the scan.
  - All matmuls bf16 (PE runs fp32 at 1/4 rate); fp32 c-state and statistics.

Host glue is O(B + H^2): sum 8 partial sumexp vectors -> log -> loss;
concatenate the 8 at column-slices.
"""

import os
import sys

for _p in ("/opt/trn_rl_repo", "/root/.axon_site/_ro/trn_rl_repo"):
    if os.path.isdir(_p) and _p not in sys.path:
        sys.path.append(_p)

import numpy as np
import ml_dtypes

import concourse.bass as bass
import concourse.mybir as mybir
from concourse.tile import TileContext
from concourse.masks import make_identity

BF16 = mybir.dt.bfloat16
F32 = mybir.dt.float32
I32 = mybir.dt.int32
AF = mybir.ActivationFunctionType
ALU = mybir.AluOpType

V = 50257
H = 512
B = 64
G = 4 * H  # 2048
NCORES = 8
ES = H // NCORES  # 64 attention-output columns per core
VS = 13 * 512  # 6656 vocab columns per core (padded; 8*6656 >= V)
VPAD = NCORES * VS
KC = H // 128  # 4 contraction chunks of the H dim


def split_multi_waits(nc):
    """Split instructions with >1 sync wait into single-wait NoOps.

    The walrus build in this container caps sync waits at 1 per instruction;
    TileContext emits instructions with several. Waits run before an
    instruction's action and engines execute their stream in order, so
    hoisting all-but-one wait onto preceding NoOps is semantically identical.
    """
    n_split = 0
    for f in nc.m.functions:
        for bb in f.blocks:
            insts = bb.instructions
            new_list = []
            for inst in insts:
                si = getattr(inst, "sync_info", None)
                if si is not None and si.on_wait is not None and len(si.on_wait) > 1:
                    waits = list(si.on_wait)
                    for j, w in enumerate(waits[:-1]):
                        n_split += 1
                        new_list.append(
                            mybir.InstNoOp(
                                name=f"{inst.name}-wsplit{j}",
                                engine=inst.engine,
                                debug=getattr(inst, "debug", None),
                                ins=[],
                                outs=[],
                                sync_info=mybir.SyncInfo(on_wait=[w], on_update=[]),
                            )
                        )
                    si.on_wait = [waits[-1]]
                new_list.append(inst)
            del insts[:]
            insts.extend(new_list)
    return n_split


def build_nc(Tn=256):
    assert Tn % 2 == 0
    CH = Tn * B // 128  # token chunks of 128 (= 2 timesteps each)
    nc = bass.Bass("TRN2", target_bir_lowering=False, debug=False, num_devices=NCORES)

    lines_cp = nc.dram_tensor("lines_cp", [128, CH], I32, kind="ExternalInput")
    emb_in_bf = nc.dram_tensor("emb_in_bf", [V, H], BF16, kind="ExternalInput")
    w_ihT = nc.dram_tensor("w_ihT", [KC, 128, G], BF16, kind="ExternalInput")
    w_hhT = nc.dram_tensor("w_hhT", [KC, 128, G], BF16, kind="ExternalInput")
    bias_bc = nc.dram_tensor("bias_bc", [128, G], BF16, kind="ExternalInput")
    emb_tgt_rows = nc.dram_tensor("emb_tgt_rows", [B, H], BF16, kind="ExternalInput")
    esel = nc.dram_tensor("esel", [KC, 128, ES], BF16, kind="ExternalInput")
    w_outT = nc.dram_tensor("w_outT", [KC, 128, VS], BF16, kind="ExternalInput")
    bout_row = nc.dram_tensor("bout_row", [1, VS], F32, kind="ExternalInput")
    w_out_tgt = nc.dram_tensor("w_out_tgt", [B, H], BF16, kind="ExternalInput")

    out_at = nc.dram_tensor("out_at", [KC, 128, ES], F32, kind="ExternalOutput")
    out_sumexp = nc.dram_tensor("out_sumexp", [B, 1], F32, kind="ExternalOutput")
    out_ztgt = nc.dram_tensor("out_ztgt", [B, 1], F32, kind="ExternalOutput")

    with TileContext(nc) as tc:
        with (
            tc.tile_pool(name="singles", bufs=1) as sing,
            tc.tile_pool(name="dram_gih", bufs=CH, space="DRAM") as dpool,
        ):
            ident = sing.tile([128, 128], BF16, tag="ident")
            make_identity(nc, ident[:])
            w_ihT_s = sing.tile([128, KC, G], BF16, tag="wih")
            w_hhT_s = sing.tile([128, KC, G], BF16, tag="whh")
            for k in range(KC):
                nc.sync.dma_start(w_ihT_s[:, k, :], w_ihT[k])
                nc.sync.dma_start(w_hhT_s[:, k, :], w_hhT[k])
            bias_s = sing.tile([128, G], BF16, tag="bias")
            nc.sync.dma_start(bias_s[:], bias_bc[:, :])
            esel_s = sing.tile([128, KC, ES], BF16, tag="esel")
            for k in range(KC):
                nc.sync.dma_start(esel_s[:, k, :], esel[k])
            lines_s = sing.tile([128, CH], I32, tag="lines")
            nc.sync.dma_start(lines_s[:], lines_cp[:, :])
            hs_store = sing.tile([B, Tn * ES], BF16, tag="hs_store")
            emb_tgt_s = sing.tile([B, H], BF16, tag="embtgt")
            nc.sync.dma_start(emb_tgt_s[:], emb_tgt_rows[:, :])
            w_out_tgt_s = sing.tile([B, H], BF16, tag="wouttgt")
            nc.sync.dma_start(w_out_tgt_s[:], w_out_tgt[:, :])
            ones_s = sing.tile([1, B], BF16, tag="ones")
            nc.vector.memset(ones_s[:], 1.0)
            hs_keep = sing.tile([B, H], BF16, tag="hs_keep")
            hsT_keep = sing.tile([128, KC, B], BF16, tag="hsT_keep")
            ht_keep = sing.tile([B, H], BF16, tag="ht_keep")

            gih_tiles = [
                dpool.tile([128, G], BF16, tag="gih", name=f"gih_{c}")
                for c in range(CH)
            ]

            # ---- phase B: gih[t] = x_t @ W_ih.T + b for all t ----
            with (
                tc.tile_pool(name="pb_sb", bufs=3) as pb,
                tc.tile_pool(name="pb_ev", bufs=2) as pbe,
                tc.tile_pool(name="pb_ps", bufs=1, space="PSUM") as pbps,
                tc.tile_pool(name="pb_psxt", bufs=2, space="PSUM") as pbpsx,
            ):
                for c in range(CH):
                    xg = pb.tile([128, H], BF16, tag="xg")
                    nc.gpsimd.indirect_dma_start(
                        out=xg[:],
                        out_offset=None,
                        in_=emb_in_bf[:],
                        in_offset=bass.IndirectOffsetOnAxis(
                            ap=lines_s[:, c : c + 1], axis=0
                        ),
                    )
                    ps_xT = pbpsx.tile([128, KC, 128], BF16, tag="psxt")
                    for k in range(KC):
                        nc.tensor.transpose(
                            ps_xT[:, k, :], xg[:, 128 * k : 128 * (k + 1)], ident[:]
                        )
                    xT = pb.tile([128, KC, 128], BF16, tag="xT")
                    nc.vector.tensor_copy(xT[:], ps_xT[:])
                    ps_g = pbps.tile([128, G], F32, tag="psg")
                    for n in range(4):
                        for k in range(KC):
                            nc.tensor.matmul(
                                ps_g[:, 512 * n : 512 * (n + 1)],
                                xT[:, k, :],
                                w_ihT_s[:, k, 512 * n : 512 * (n + 1)],
                                start=(k == 0),
                                stop=(k == KC - 1),
                            )
                    ev = pbe.tile([128, G], BF16, tag="ev")
                    nc.vector.tensor_add(ev[:], ps_g[:], bias_s[:])
                    nc.sync.dma_start(gih_tiles[c][:], ev[:])

            # ---- phases C+D: LSTM scan + decoder step ----
            with (
                tc.tile_pool(name="sc_state", bufs=2) as scs,
                tc.tile_pool(name="sc_work", bufs=2) as scw,
            ):
                with (
                    tc.tile_pool(name="sc_ps_g", bufs=1, space="PSUM") as scpsg,
                    tc.tile_pool(name="sc_ps_h", bufs=2, space="PSUM") as scpsh,
                    tc.tile_pool(name="sc_ps_sl", bufs=2, space="PSUM") as scpssl,
                ):
                    c_cur = scs.tile([B, H], F32, tag="c_state")
                    nc.vector.memset(c_cur[:], 0.0)
                    hT_cur = scs.tile([128, KC, B], BF16, tag="hT")
                    nc.vector.memset(hT_cur[:], 0.0)

                    def lstm_step(gih_src, x_lhsT, t_store):
                        nonlocal c_cur, hT_cur
                        ps_g = scpsg.tile([B, G], F32, tag="ps_gate")
                        groups = [(hT_cur[:, k, :], w_hhT_s, k) for k in range(KC)]
                        if x_lhsT is not None:
                            groups += [
                                (x_lhsT[:, k, :], w_ihT_s, k) for k in range(KC)
                            ]
                        ng = len(groups)
                        for n in range(4):
                            for gi, (lhsT, wsrc, k) in enumerate(groups):
                                nc.tensor.matmul(
                                    ps_g[:, 512 * n : 512 * (n + 1)],
                                    lhsT,
                                    wsrc[:, k, 512 * n : 512 * (n + 1)],
                                    start=(gi == 0),
                                    stop=(gi == ng - 1),
                                )
                        z = scw.tile([B, G], BF16, tag="z")
                        if gih_src is not None:
                            nc.vector.tensor_add(z[:], ps_g[:], gih_src)
                        else:
                            nc.vector.tensor_add(z[:], ps_g[:], bias_s[0:B, :])
                        gates = scw.tile([B, G], BF16, tag="gates")
                        nc.scalar.activation(gates[:, 0:1024], z[:, 0:1024], AF.Sigmoid)
                        nc.scalar.activation(
                            gates[:, 1024:1536], z[:, 1024:1536], AF.Tanh
                        )
                        nc.scalar.activation(
                            gates[:, 1536:2048], z[:, 1536:2048], AF.Sigmoid
                        )
                        i_ = gates[:, 0:512]
                        f_ = gates[:, 512:1024]
                        g_ = gates[:, 1024:1536]
                        o_ = gates[:, 1536:2048]
                        t1 = scw.tile([B, H], F32, tag="t1")
                        nc.vector.tensor_mul(t1[:], i_, g_)
                        c_new = scs.tile([B, H], F32, tag="c_state")
                        nc.vector.tensor_mul(c_new[:], f_, c_cur[:])
                        nc.vector.tensor_add(c_new[:], c_new[:], t1[:])
                        tc_t = scw.tile([B, H], BF16, tag="tanh_c")
                        nc.scalar.activation(tc_t[:], c_new[:], AF.Tanh)
                        h = scw.tile([B, H], BF16, tag="h")
                        nc.vector.tensor_mul(h[:], o_, tc_t[:])
                        ps_hT = scpsh.tile([128, KC, B], BF16, tag="ps_hT")
                        for k in range(KC):
                            nc.tensor.transpose(
                                ps_hT[:, k, :],
                                h[:, 128 * k : 128 * (k + 1)],
                                ident[0:B, 0:B],
                            )
                        hT_new = scs.tile([128, KC, B], BF16, tag="hT")
                        nc.vector.tensor_copy(hT_new[:], ps_hT[:])
                        if t_store >= 0:
                            ps_sl = scpssl.tile([B, ES], F32, tag="ps_sl")
                            for k in range(KC):
                                nc.tensor.matmul(
                                    ps_sl[:],
                                    hT_new[:, k, :],
                                    esel_s[:, k, :],
                                    start=(k == 0),
                                    stop=(k == KC - 1),
                                )
                            nc.vector.tensor_copy(
                                hs_store[:, ES * t_store : ES * (t_store + 1)],
                                ps_sl[:],
                            )
                        c_cur = c_new
                        hT_cur = hT_new
                        return h

                    h_nat = None
                    for t in range(Tn):
                        ch, half = divmod(t, 2)
                        gih = scw.tile([B, G], BF16, tag="gih", bufs=3)
                        nc.sync.dma_start(
                            gih[:], gih_tiles[ch][64 * half : 64 * half + 64, :]
                        )
                        h_nat = lstm_step(gih[:], None, t)

                    nc.vector.tensor_copy(hs_keep[:], h_nat[:])
                    nc.vector.tensor_copy(hsT_keep[:], hT_cur[:])

                    ps_xd = scpsh.tile([128, KC, B], BF16, tag="ps_hT")
                    for k in range(KC):
                        nc.tensor.transpose(
                            ps_xd[:, k, :],
                            emb_tgt_s[:, 128 * k : 128 * (k + 1)],
                            ident[0:B, 0:B],
                        )
                    xdT = scw.tile([128, KC, B], BF16, tag="xdT", bufs=1)
                    nc.vector.tensor_copy(xdT[:], ps_xd[:])
                    ht_nat = lstm_step(None, xdT, -1)
                    nc.vector.tensor_copy(ht_keep[:], ht_nat[:])

                    # z_at_target = <hs, W_out[tgt]> rowwise
                    ztg_scr = scw.tile([B, H], F32, tag="ztg_scr", bufs=1)
                    ztg = scw.tile([B, 1], F32, tag="ztg", bufs=1)
                    nc.vector.tensor_mul(ztg_scr[:], hs_keep[:], w_out_tgt_s[:])
                    nc.vector.tensor_reduce(
                        out=ztg[:], in_=ztg_scr[:], axis=mybir.AxisListType.X, op=ALU.add
                    )
                    nc.sync.dma_start(out_ztgt[:, :], ztg[:])

                # ---- phase E: attention slice ----
                TG = min(32, Tn)
                n_tg = Tn // TG
                MMW = min(512, TG * ES)
                with (
                    tc.tile_pool(name="at_ps", bufs=2, space="PSUM") as atps,
                    tc.tile_pool(name="at_sb", bufs=2) as atsb,
                    tc.tile_pool(name="at_acc", bufs=1) as atacc,
                ):
                    acc = atacc.tile([128, KC, ES], F32, tag="acc")
                    exp0 = atacc.tile([128, KC, ES], F32, tag="exp0")
                    for m in range(KC):
                        for g in range(n_tg):
                            ps_s = atps.tile([128, TG * ES], F32, tag="ps_s")
                            for q in range(TG * ES // MMW):
                                nc.tensor.matmul(
                                    ps_s[:, MMW * q : MMW * (q + 1)],
                                    ht_keep[:, 128 * m : 128 * (m + 1)],
                                    hs_store[
                                        :,
                                        TG * ES * g
                                        + MMW * q : TG * ES * g
                                        + MMW * (q + 1),
                                    ],
                                    start=True,
                                    stop=True,
                                )
                            ex = atsb.tile([128, TG * ES], F32, tag="ex")
                            nc.scalar.activation(ex[:], ps_s[:], AF.Exp)
                            if g == 0:
                                nc.vector.tensor_copy(exp0[:, m, :], ex[:, 0:ES])
                            w = TG * ES // 2
                            while w >= ES:
                                nc.vector.tensor_add(
                                    ex[:, 0:w], ex[:, 0:w], ex[:, w : 2 * w]
                                )
                                w //= 2
                            if g == 0:
                                nc.vector.tensor_copy(acc[:, m, :], ex[:, 0:ES])
                            else:
                                nc.vector.tensor_add(
                                    acc[:, m, :], acc[:, m, :], ex[:, 0:ES]
                                )
                    rec = atsb.tile([128, KC, ES], F32, tag="rec")
                    nc.vector.reciprocal(rec[:], acc[:])
                    at_t = atsb.tile([128, KC, ES], F32, tag="at")
                    nc.vector.tensor_mul(at_t[:], exp0[:], rec[:])
                    for m in range(KC):
                        nc.sync.dma_start(out_at[m], at_t[:, m, :])

                # ---- phase F: vocab-shard sumexp of logits ----
                with (
                    tc.tile_pool(name="lg_ps", bufs=4, space="PSUM") as lgps,
                    tc.tile_pool(name="lg_sb", bufs=2) as lgsb,
                    tc.tile_pool(name="lg_acc", bufs=1) as lgacc,
                ):
                    nchunk = VS // 512
                    w_outT_s = lgacc.tile([128, KC, VS], BF16, tag="wout")
                    for k in range(KC):
                        nc.sync.dma_start(w_outT_s[:, k, :], w_outT[k])
                    bout_s = lgacc.tile([1, VS], F32, tag="bout")
                    nc.sync.dma_start(bout_s[:], bout_row[:, :])
                    sums = lgacc.tile([B, nchunk], F32, tag="sums")
                    for cix in range(nchunk):
                        ps_l = lgps.tile([B, 512], F32, tag="ps_l")
                        for k in range(KC):
                            nc.tensor.matmul(
                                ps_l[:],
                                hsT_keep[:, k, :],
                                w_outT_s[:, k, 512 * cix : 512 * (cix + 1)],
                                start=(k == 0),
                                stop=False,
                            )
                        bz = lgsb.tile([1, 512], BF16, tag="bz")
                        nc.vector.tensor_copy(
                            bz[:], bout_s[:, 512 * cix : 512 * (cix + 1)]
                        )
                        nc.tensor.matmul(
                            ps_l[:], ones_s[:], bz[:], start=False, stop=True
                        )
                        scr = lgsb.tile([B, 512], BF16, tag="scr")
                        nc.scalar.activation(
                            scr[:], ps_l[:], AF.Exp, accum_out=sums[:, cix : cix + 1]
                        )
                    tot = lgacc.tile([B, 1], F32, tag="tot")
                    nc.vector.tensor_reduce(
                        out=tot[:], in_=sums[:], axis=mybir.AxisListType.X, op=ALU.add
                    )
                    nc.sync.dma_start(out_sumexp[:, :], tot[:])

    return nc


def prep_inputs(input_lines, target_lines, emb_in, emb_tgt, W_ih, W_hh, b_ih, b_hh,
                W_out, b_out, Tn=256):
    bf = ml_dtypes.bfloat16
    input_lines = np.asarray(input_lines)[:Tn]
    tgt0 = np.asarray(target_lines)[0].astype(np.int64)

    CH = Tn * B // 128
    lines_flat = np.asarray(input_lines).reshape(-1).astype(np.int32)
    lines_cp = np.ascontiguousarray(lines_flat.reshape(CH, 128).T)

    emb_in_bf = np.ascontiguousarray(np.asarray(emb_in), dtype=bf)
    w_ihT = np.ascontiguousarray(np.asarray(W_ih).T.astype(bf).reshape(KC, 128, G))
    w_hhT = np.ascontiguousarray(np.asarray(W_hh).T.astype(bf).reshape(KC, 128, G))
    bias = (np.asarray(b_ih) + np.asarray(b_hh)).astype(np.float32)
    bias_bc = np.ascontiguousarray(np.broadcast_to(bias.astype(bf), (128, G)))
    emb_tgt_rows = np.ascontiguousarray(np.asarray(emb_tgt)[tgt0], dtype=bf)
    w_out_tgt = np.ascontiguousarray(np.asarray(W_out)[tgt0], dtype=bf)

    W_outT_pad = np.zeros((H, VPAD), dtype=bf)
    W_outT_pad[:, :V] = np.asarray(W_out).T.astype(bf)
    bout_pad = np.full((VPAD,), -1e30, dtype=np.float32)
    bout_pad[:V] = np.asarray(b_out).astype(np.float32)

    in_maps = []
    for k in range(NCORES):
        esel_k = np.zeros((H, ES), dtype=bf)
        for e in range(ES):
            esel_k[ES * k + e, e] = 1.0
        in_maps.append(
            dict(
                lines_cp=lines_cp,
                emb_in_bf=emb_in_bf,
                w_ihT=w_ihT,
                w_hhT=w_hhT,
                bias_bc=bias_bc,
                emb_tgt_rows=emb_tgt_rows,
                esel=np.ascontiguousarray(esel_k.reshape(KC, 128, ES)),
                w_outT=np.ascontiguousarray(
                    W_outT_pad[:, VS * k : VS * (k + 1)].reshape(KC, 128, VS)
                ),
                bout_row=np.ascontiguousarray(
                    bout_pad[VS * k : VS * (k + 1)].reshape(1, VS)
                ),
                w_out_tgt=w_out_tgt,
            )
        )
    host_ctx = dict(tgt0=tgt0, b_out=np.asarray(b_out).astype(np.float64))
    return in_maps, host_ctx


def postprocess(results, host_ctx):
    tgt0 = host_ctx["tgt0"]
    b_out = host_ctx["b_out"]
    total_sumexp = np.zeros(B, dtype=np.float64)
    for r in results:
        total_sumexp += r["out_sumexp"][:, 0].astype(np.float64)
    ztgt = results[0]["out_ztgt"][:, 0].astype(np.float64) + b_out[tgt0]
    loss = float(np.mean(np.log(total_sumexp) - ztgt))
    at = np.zeros((H, H), dtype=np.float32)
    for k, r in enumerate(results):
        at[:, ES * k : ES * (k + 1)] = r["out_at"].reshape(H, ES)
    return np.float32(loss), at


# ---------------- cached runner ----------------

_RUNNER = None


class _Runner:
    """Builds the Bass program once and keeps a reusable sharded-jit callable
    (mirrors concourse.bass2jax.run_bass_via_pjrt, but reusable so repeat
    calls don't recompile and inputs can stay device-resident for timing)."""

    def __init__(self, Tn=256):
        import jax
        from jax.sharding import Mesh, PartitionSpec
        from jax.experimental.shard_map import shard_map
        from concourse import bass2jax

        bass2jax.install_neuronx_cc_hook()
        self.jax = jax
        self.PartitionSpec = PartitionSpec
        nc = build_nc(Tn=Tn)
        split_multi_waits(nc)
        self.nc = nc

        partition_name = (
            nc.partition_id_tensor.name if nc.partition_id_tensor else None
        )
        in_names, out_names, out_avals, zero_outs = [], [], [], []
        for alloc in nc.m.functions[0].allocations:
            if not isinstance(alloc, mybir.MemoryLocationSet):
                continue
            name = alloc.memorylocations[0].name
            if alloc.kind == "ExternalInput":
                if name != partition_name:
                    in_names.append(name)
            elif alloc.kind == "ExternalOutput":
                shape = tuple(alloc.tensor_shape)
                dt = mybir.dt.np(alloc.dtype)
                out_names.append(name)
                out_avals.append(jax.core.ShapedArray(shape, dt))
                zero_outs.append(np.zeros(shape, dt))
        self.in_names = list(in_names)
        self.out_names = out_names
        self.zero_outs = zero_outs
        n_params = len(in_names)
        n_outs = len(out_names)
        all_in_names = in_names + out_names
        if partition_name is not None:
            all_in_names = all_in_names + [partition_name]

        def _body(*args):
            operands = list(args)
            if partition_name is not None:
                operands.append(bass2jax.partition_id_tensor())
            outs = bass2jax._bass_exec_p.bind(
                *operands,
                out_avals=tuple(out_avals),
                in_names=tuple(all_in_names),
                out_names=tuple(out_names),
                lowering_input_output_aliases=(),
                sim_require_finite=True,
                sim_require_nnan=True,
                nc=nc,
            )
            return tuple(outs)

        devices = jax.devices()[:NCORES]
        self.mesh = Mesh(np.asarray(devices), ("core",))
        in_specs = (PartitionSpec("core"),) * (n_params + n_outs)
        out_specs = (PartitionSpec("core"),) * n_outs
        self.donate = tuple(range(n_params, n_params + n_outs))
        self.sharded = jax.jit(
            shard_map(
                _body,
                mesh=self.mesh,
                in_specs=in_specs,
                out_specs=out_specs,
                check_rep=False,
            ),
            donate_argnums=self.donate,
            keep_unused=True,
        )

    def concat_inputs(self, in_maps):
        return [
            np.concatenate([np.asarray(in_maps[c][n]) for c in range(NCORES)], axis=0)
            for n in self.in_names
        ]

    def _zeros(self):
        return [
            np.zeros((NCORES * z.shape[0], *z.shape[1:]), z.dtype)
            for z in self.zero_outs
        ]

    def execute(self, concat_in):
        out_arrs = self.sharded(*concat_in, *self._zeros())
        results = [{} for _ in range(NCORES)]
        for i, name in enumerate(self.out_names):
            full = np.asarray(out_arrs[i])
            per = full.reshape((NCORES, full.shape[0] // NCORES) + full.shape[1:])
            for c in range(NCORES):
                results[c][name] = per[c]
        return results

    def run(self, in_maps):
        return self.execute(self.concat_inputs(in_maps))


def _get_runner():
    global _RUNNER
    if _RUNNER is None:
        _RUNNER = _Runner(Tn=256)
    return _RUNNER


def kernel(**inputs):
    r = _get_runner()
    in_maps, host_ctx = prep_inputs(**inputs)
    results = r.run(in_maps)
    return postprocess(results, host_ctx)


def time_kernel_ns(inputs, iters=5):
    """Median wall time per execution with device-resident inputs."""
    import time as _time

    r = _get_runner()
    jax = r.jax
    in_maps, _ = prep_inputs(**inputs)
    concat_in = r.concat_inputs(in_maps)
    from jax.sharding import NamedSharding

    sh = NamedSharding(r.mesh, r.PartitionSpec("core"))
    dev_in = [jax.device_put(a, sh) for a in concat_in]
    # warmup (compile + first exec)
    jax.block_until_ready(r.sharded(*dev_in, *r._zeros()))
    times = []
    for _ in range(iters):
        t0 = _time.perf_counter()
        jax.block_until_ready(r.sharded(*dev_in, *r._zeros()))
        times.append(_time.perf_counter() - t0)
    med = sorted(times)[len(times) // 2]
    return int(med * 1e9)


# revision 12
# speedup vs baseline: 31.3074x; 31.3074x over previous
"""nn_Encoder_Decoder kernel for 8 trn2 NeuronCores (Bass/Tile, SPMD).

Contract: kernel(**inputs) takes the FULL unsharded inputs of
reference.setup_inputs() and returns the FULL output (loss scalar, at[512,512]).

Sharding strategy (per spec hint, adapted):
  - The 256-step LSTM scan is replicated on all 8 cores: it is latency-bound
    (per-step time is independent of batch), and the batch-contracted
    attention einsum needs full-batch hs on every core anyway.
  - Per-core shards: 1/8 of the attention output columns (one-hot-selected
    e-slice of hs kept during the scan) and 1/8 of the vocab for the
    output-projection softmax denominator (tensor parallel over [V,H]).
  - The x-side gate projections (x_t @ W_ih.T + b) are interleaved with the
    scan and accumulated straight into the gate PSUM (two timesteps share a
    [128, 2048] psum pair; the odd step lands on partitions 64-127 via
    tile_position col placement), so no staging and no separate add pass.
  - Recurrent/x projections run in fp8-e4m3 DoubleRow (h pre-scaled by 64,
    weights by 16; the 1/1024 washout rides the activation's free affine).
    fp32 c-state and all softmax/attention statistics.

Host glue is O(B + H^2): sum 8 partial sumexp vectors -> log -> loss;
concatenate the 8 at column-slices.
"""

import os
import sys

for _p in ("/opt/trn_rl_repo", "/root/.axon_site/_ro/trn_rl_repo"):
    if os.path.isdir(_p) and _p not in sys.path:
        sys.path.append(_p)

import numpy as np
import ml_dtypes

import concourse.bass as bass
import concourse.mybir as mybir
from concourse.tile import TileContext
from concourse.masks import make_identity

BF16 = mybir.dt.bfloat16
F32 = mybir.dt.float32
I32 = mybir.dt.int32
F8 = mybir.dt.float8e4
AF = mybir.ActivationFunctionType
ALU = mybir.AluOpType
DR = mybir.MatmulPerfMode.DoubleRow

V = 50257
H = 512
B = 64
G = 4 * H  # 2048
NCORES = 8
ES = H // NCORES  # 64 attention-output columns per core
VS = 13 * 512  # 6656 vocab columns per core (padded; 8*6656 >= V)
VPAD = NCORES * VS
KC = H // 128  # 4 bf16 contraction chunks of the H dim
KK = 2  # fp8 DoubleRow contraction chunks (2 x 256)
HS = 64.0  # fp8 scale on h / x
WS = 16.0  # fp8 scale on W_ih / W_hh
GS = HS * WS  # resulting gate pre-activation scale in PSUM


def split_multi_waits(nc):
    """Split instructions with >1 sync wait into single-wait NoOps.

    The walrus build in this container caps sync waits at 1 per instruction;
    TileContext emits instructions with several. Waits run before an
    instruction's action and engines execute their stream in order, so
    hoisting all-but-one wait onto preceding NoOps is semantically identical.
    """
    n_split = 0
    for f in nc.m.functions:
        for bb in f.blocks:
            insts = bb.instructions
            new_list = []
            for inst in insts:
                si = getattr(inst, "sync_info", None)
                if si is not None and si.on_wait is not None and len(si.on_wait) > 1:
                    waits = list(si.on_wait)
                    for j, w in enumerate(waits[:-1]):
                        n_split += 1
                        new_list.append(
                            mybir.InstNoOp(
                                name=f"{inst.name}-wsplit{j}",
                                engine=inst.engine,
                                debug=getattr(inst, "debug", None),
                                ins=[],
                                outs=[],
                                sync_info=mybir.SyncInfo(on_wait=[w], on_update=[]),
                            )
                        )
                    si.on_wait = [waits[-1]]
                new_list.append(inst)
            del insts[:]
            insts.extend(new_list)
    return n_split


def build_nc(Tn=256, reps=1):
    assert Tn % 2 == 0
    CH = Tn * B // 128  # token chunks of 128 (= 2 timesteps each)
    nc = bass.Bass("TRN2", target_bir_lowering=False, debug=False, num_devices=NCORES)

    lines_cp = nc.dram_tensor("lines_cp", [128, CH], I32, kind="ExternalInput")
    emb_in_bf = nc.dram_tensor("emb_in_bf", [V, H], BF16, kind="ExternalInput")
    w_ihT = nc.dram_tensor("w_ihT", [KK, 128, 2, G], F8, kind="ExternalInput")
    w_hhT = nc.dram_tensor("w_hhT", [KK, 128, 2, G], F8, kind="ExternalInput")
    bias_row = nc.dram_tensor("bias_row", [1, G], BF16, kind="ExternalInput")
    emb_tgt_rows = nc.dram_tensor("emb_tgt_rows", [B, H], BF16, kind="ExternalInput")
    esel = nc.dram_tensor("esel", [KK, 128, 2, ES], F8, kind="ExternalInput")
    w_outT = nc.dram_tensor("w_outT", [KC, 128, VS], BF16, kind="ExternalInput")
    bout_row = nc.dram_tensor("bout_row", [1, VS], F32, kind="ExternalInput")
    w_out_tgt = nc.dram_tensor("w_out_tgt", [B, H], BF16, kind="ExternalInput")

    out_at = nc.dram_tensor("out_at", [KC, 128, ES], F32, kind="ExternalOutput")
    out_sumexp = nc.dram_tensor("out_sumexp", [B, 1], F32, kind="ExternalOutput")
    out_ztgt = nc.dram_tensor("out_ztgt", [B, 1], F32, kind="ExternalOutput")

    with TileContext(nc) as tc:
        with tc.tile_pool(name="singles", bufs=1) as sing:
            ident = sing.tile([128, 128], BF16, tag="ident")
            make_identity(nc, ident[:])
            w_ihT_s = sing.tile([128, KK, 2, G], F8, tag="wih")
            w_hhT_s = sing.tile([128, KK, 2, G], F8, tag="whh")
            for kk in range(KK):
                nc.sync.dma_start(w_ihT_s[:, kk, :, :], w_ihT[kk])
                nc.sync.dma_start(w_hhT_s[:, kk, :, :], w_hhT[kk])
            bias_s = sing.tile([1, G], BF16, tag="bias")
            nc.sync.dma_start(bias_s[:], bias_row[:, :])
            esel_s = sing.tile([128, KK, 2, ES], F8, tag="esel")
            for kk in range(KK):
                nc.sync.dma_start(esel_s[:, kk, :, :], esel[kk])
            lines_s = sing.tile([128, CH], I32, tag="lines")
            nc.sync.dma_start(lines_s[:], lines_cp[:, :])
            hs_store = sing.tile([B, Tn * ES], BF16, tag="hs_store")
            emb_tgt_s = sing.tile([B, H], BF16, tag="embtgt")
            nc.sync.dma_start(emb_tgt_s[:], emb_tgt_rows[:, :])
            w_out_tgt_s = sing.tile([B, H], BF16, tag="wouttgt")
            nc.sync.dma_start(w_out_tgt_s[:], w_out_tgt[:, :])
            ones_s = sing.tile([1, 128], BF16, tag="ones")
            nc.vector.memset(ones_s[:], 1.0)
            hs_keep = sing.tile([B, H], BF16, tag="hs_keep")
            hsT_keep = sing.tile([128, KC, B], BF16, tag="hsT_keep")
            ht_keep = sing.tile([B, H], BF16, tag="ht_keep")

            for _rep in range(reps):
                # ---- scan with interleaved x-side projections ----
                with (
                    tc.tile_pool(name="sc_state", bufs=2) as scs,
                    tc.tile_pool(name="sc_work", bufs=2) as scw,
                    tc.tile_pool(name="pb_sb", bufs=3) as pb,
                ):
                    with (
                        tc.tile_pool(name="sc_ps_big", bufs=3, space="PSUM") as bigp,
                        tc.tile_pool(name="sc_ps_aux", bufs=2, space="PSUM") as auxp,
                    ):
                        c_cur = scs.tile([B, H], F32, tag="c_state")
                        nc.vector.memset(c_cur[:], 0.0)
                        hT_cur = scs.tile([128, KK, 2, B], F8, tag="hT")
                        nc.vector.memset(hT_cur[:], 0.0)
                        last_ps_hT = None

                        def x_side(xT_lhsT, col0):
                            """Emit x@W_ih.T + b for ONE step (batch cols
                            [col0, col0+64) of the chunk's xT) into two fresh
                            [64,1024] psum tiles (starts the accum group).
                            xT_lhsT: fp8 [128, KK, 2, >=col0+64] (x HS)."""
                            bigA = bigp.tile([B, 1024], F32, tag="big")
                            bigB = bigp.tile([B, 1024], F32, tag="big")
                            for bt, ns in ((bigA, (0, 1)), (bigB, (2, 3))):
                                for j, n in enumerate(ns):
                                    sl = bt[:, 512 * j : 512 * (j + 1)]
                                    for kk in range(KK):
                                        nc.tensor.matmul(
                                            sl,
                                            xT_lhsT[:, kk, :, col0 : col0 + B],
                                            w_ihT_s[:, kk, :, 512 * n : 512 * (n + 1)],
                                            start=(kk == 0),
                                            stop=False,
                                            perf_mode=DR,
                                        )
                                    nc.tensor.matmul(
                                        sl,
                                        ones_s[:, 0:B],
                                        bias_s[:, 512 * n : 512 * (n + 1)],
                                        start=False,
                                        stop=False,
                                    )
                            return bigA, bigB

                        def lstm_step(bigA, bigB, t_store):
                            """One LSTM cell step; gates in psum rows [0, 64)
                            of bigA (i,f) / bigB (g,o)."""
                            nonlocal c_cur, hT_cur, last_ps_hT
                            for bt, ns in ((bigA, (0, 1)), (bigB, (2, 3))):
                                for j, n in enumerate(ns):
                                    sl = bt[:, 512 * j : 512 * (j + 1)]
                                    for kk in range(KK):
                                        nc.tensor.matmul(
                                            sl,
                                            hT_cur[:, kk, :, :],
                                            w_hhT_s[:, kk, :, 512 * n : 512 * (n + 1)],
                                            start=False,
                                            stop=(kk == KK - 1),
                                            perf_mode=DR,
                                        )
                            gates = scw.tile([B, G], BF16, tag="gates")
                            nc.scalar.activation(
                                gates[:, 0:1024], bigA[:, :], AF.Sigmoid,
                                scale=1.0 / GS,
                            )
                            nc.scalar.activation(
                                gates[:, 1024:1536], bigB[:, 0:512], AF.Tanh,
                                scale=1.0 / GS,
                            )
                            nc.scalar.activation(
                                gates[:, 1536:2048], bigB[:, 512:1024], AF.Sigmoid,
                                scale=1.0 / GS,
                            )
                            i_ = gates[:, 0:512]
                            f_ = gates[:, 512:1024]
                            g_ = gates[:, 1024:1536]
                            o_ = gates[:, 1536:2048]
                            t1 = scw.tile([B, H], F32, tag="t1")
                            nc.gpsimd.tensor_mul(t1[:], i_, g_)
                            c_new = scs.tile([B, H], F32, tag="c_state")
                            nc.vector.tensor_mul(c_new[:], f_, c_cur[:])
                            nc.vector.tensor_add(c_new[:], c_new[:], t1[:])
                            tc_t = scw.tile([B, H], BF16, tag="tanh_c")
                            nc.scalar.activation(tc_t[:], c_new[:], AF.Tanh)
                            h = scw.tile([B, H], BF16, tag="h")
                            nc.gpsimd.tensor_mul(h[:], o_, tc_t[:])
                            ps_hT = auxp.tile(
                                [128, KC, 128], BF16, tag="aux", name="ps_hT"
                            )
                            last_ps_hT = ps_hT
                            for k in range(KC):
                                nc.tensor.transpose(
                                    ps_hT[:, k, 0:B],
                                    h[:, 128 * k : 128 * (k + 1)],
                                    ident[0:B, 0:B],
                                )
                            hT_new = scs.tile([128, KK, 2, B], F8, tag="hT")
                            nc.vector.tensor_scalar_mul(hT_new[:], ps_hT[:, :, 0:B], HS)
                            if t_store >= 0:
                                ps_sl = auxp.tile([B, ES], F32, tag="aux", name="ps_sl")
                                for kk in range(KK):
                                    nc.tensor.matmul(
                                        ps_sl[:],
                                        hT_new[:, kk, :, :],
                                        esel_s[:, kk, :, :],
                                        start=(kk == 0),
                                        stop=(kk == KK - 1),
                                        perf_mode=DR,
                                    )
                                nc.vector.tensor_scalar_mul(
                                    hs_store[:, ES * t_store : ES * (t_store + 1)],
                                    ps_sl[:],
                                    1.0 / HS,
                                )
                            c_cur = c_new
                            hT_cur = hT_new
                            return h

                        h_nat = None
                        for c in range(CH):
                            xg = pb.tile([128, H], BF16, tag="xg")
                            nc.gpsimd.indirect_dma_start(
                                out=xg[:],
                                out_offset=None,
                                in_=emb_in_bf[:],
                                in_offset=bass.IndirectOffsetOnAxis(
                                    ap=lines_s[:, c : c + 1], axis=0
                                ),
                            )
                            ps_xT = auxp.tile(
                                [128, KC, 128], BF16, tag="aux", name="ps_xT"
                            )
                            for k in range(KC):
                                nc.tensor.transpose(
                                    ps_xT[:, k, :],
                                    xg[:, 128 * k : 128 * (k + 1)],
                                    ident[:],
                                )
                            xT = pb.tile([128, KK, 2, 128], F8, tag="xT")
                            nc.vector.tensor_scalar_mul(xT[:], ps_xT[:], HS)
                            for half in range(2):
                                bigA, bigB = x_side(xT, B * half)
                                h_nat = lstm_step(bigA, bigB, 2 * c + half)

                        nc.vector.tensor_copy(hs_keep[:], h_nat[:])
                        nc.vector.tensor_copy(hsT_keep[:], last_ps_hT[:, :, 0:B])

                        # decoder step (x = emb_tgt rows, M=64)
                        ps_xd = auxp.tile([128, KC, 128], BF16, tag="aux", name="ps_xd")
                        for k in range(KC):
                            nc.tensor.transpose(
                                ps_xd[:, k, 0:B],
                                emb_tgt_s[:, 128 * k : 128 * (k + 1)],
                                ident[0:B, 0:B],
                            )
                        xdT = scw.tile([128, KK, 2, B], F8, tag="xdT", bufs=1)
                        nc.vector.tensor_scalar_mul(xdT[:], ps_xd[:, :, 0:B], HS)
                        bigA, bigB = x_side(xdT, 0)
                        ht_nat = lstm_step(bigA, bigB, -1)
                        nc.vector.tensor_copy(ht_keep[:], ht_nat[:])

                        # z_at_target = <hs, W_out[tgt]> rowwise
                        ztg_scr = scw.tile([B, H], F32, tag="ztg_scr", bufs=1)
                        ztg = scw.tile([B, 1], F32, tag="ztg", bufs=1)
                        nc.vector.tensor_mul(ztg_scr[:], hs_keep[:], w_out_tgt_s[:])
                        nc.vector.tensor_reduce(
                            out=ztg[:], in_=ztg_scr[:], axis=mybir.AxisListType.X,
                            op=ALU.add,
                        )
                        nc.sync.dma_start(out_ztgt[:, :], ztg[:])

                    # ---- attention slice ----
                    TG = min(32, Tn)
                    n_tg = Tn // TG
                    MMW = min(512, TG * ES)
                    with (
                        tc.tile_pool(name="at_ps", bufs=2, space="PSUM") as atps,
                        tc.tile_pool(name="at_sb", bufs=2) as atsb,
                        tc.tile_pool(name="at_acc", bufs=1) as atacc,
                    ):
                        acc = atacc.tile([128, KC, ES], F32, tag="acc")
                        exp0 = atacc.tile([128, KC, ES], F32, tag="exp0")
                        for m in range(KC):
                            for g in range(n_tg):
                                ps_s = atps.tile([128, TG * ES], F32, tag="ps_s")
                                for q in range(TG * ES // MMW):
                                    nc.tensor.matmul(
                                        ps_s[:, MMW * q : MMW * (q + 1)],
                                        ht_keep[:, 128 * m : 128 * (m + 1)],
                                        hs_store[
                                            :,
                                            TG * ES * g
                                            + MMW * q : TG * ES * g
                                            + MMW * (q + 1),
                                        ],
                                        start=True,
                                        stop=True,
                                    )
                                ex = atsb.tile([128, TG * ES], F32, tag="ex")
                                nc.scalar.activation(ex[:], ps_s[:], AF.Exp)
                                if g == 0:
                                    nc.vector.tensor_copy(exp0[:, m, :], ex[:, 0:ES])
                                w = TG * ES // 2
                                while w >= ES:
                                    nc.vector.tensor_add(
                                        ex[:, 0:w], ex[:, 0:w], ex[:, w : 2 * w]
                                    )
                                    w //= 2
                                if g == 0:
                                    nc.vector.tensor_copy(acc[:, m, :], ex[:, 0:ES])
                                else:
                                    nc.vector.tensor_add(
                                        acc[:, m, :], acc[:, m, :], ex[:, 0:ES]
                                    )
                        rec = atsb.tile([128, KC, ES], F32, tag="rec")
                        nc.vector.reciprocal(rec[:], acc[:])
                        at_t = atsb.tile([128, KC, ES], F32, tag="at")
                        nc.vector.tensor_mul(at_t[:], exp0[:], rec[:])
                        for m in range(KC):
                            nc.sync.dma_start(out_at[m], at_t[:, m, :])

                    # ---- vocab-shard sumexp of logits ----
                    with (
                        tc.tile_pool(name="lg_ps", bufs=4, space="PSUM") as lgps,
                        tc.tile_pool(name="lg_sb", bufs=2) as lgsb,
                        tc.tile_pool(name="lg_acc", bufs=1) as lgacc,
                    ):
                        nchunk = VS // 512
                        w_outT_s = lgacc.tile([128, KC, VS], BF16, tag="wout")
                        for k in range(KC):
                            nc.sync.dma_start(w_outT_s[:, k, :], w_outT[k])
                        bout_s = lgacc.tile([1, VS], F32, tag="bout")
                        nc.sync.dma_start(bout_s[:], bout_row[:, :])
                        sums = lgacc.tile([B, nchunk], F32, tag="sums")
                        for cix in range(nchunk):
                            ps_l = lgps.tile([B, 512], F32, tag="ps_l")
                            for k in range(KC):
                                nc.tensor.matmul(
                                    ps_l[:],
                                    hsT_keep[:, k, :],
                                    w_outT_s[:, k, 512 * cix : 512 * (cix + 1)],
                                    start=(k == 0),
                                    stop=False,
                                )
                            bz = lgsb.tile([1, 512], BF16, tag="bz")
                            nc.vector.tensor_copy(
                                bz[:], bout_s[:, 512 * cix : 512 * (cix + 1)]
                            )
                            nc.tensor.matmul(
                                ps_l[:], ones_s[:, 0:B], bz[:], start=False, stop=True
                            )
                            scr = lgsb.tile([B, 512], BF16, tag="scr")
                            nc.scalar.activation(
                                scr[:], ps_l[:], AF.Exp,
                                accum_out=sums[:, cix : cix + 1],
                            )
                        tot = lgacc.tile([B, 1], F32, tag="tot")
                        nc.vector.tensor_reduce(
                            out=tot[:], in_=sums[:], axis=mybir.AxisListType.X,
                            op=ALU.add,
                        )
                        nc.sync.dma_start(out_sumexp[:, :], tot[:])

    return nc


def prep_inputs(input_lines, target_lines, emb_in, emb_tgt, W_ih, W_hh, b_ih, b_hh,
                W_out, b_out, Tn=256):
    bf = ml_dtypes.bfloat16
    f8 = ml_dtypes.float8_e4m3fn
    input_lines = np.asarray(input_lines)[:Tn]
    tgt0 = np.asarray(target_lines)[0].astype(np.int64)

    CH = Tn * B // 128
    lines_flat = np.asarray(input_lines).reshape(-1).astype(np.int32)
    lines_cp = np.ascontiguousarray(lines_flat.reshape(CH, 128).T)

    def dr_layout(WT):
        # [H, cols] -> [KK, 128, 2, cols] with j = 256*kk + 128*ko + ki
        cols = WT.shape[1]
        return np.ascontiguousarray(WT.reshape(KK, 2, 128, cols).transpose(0, 2, 1, 3))

    emb_in_bf = np.ascontiguousarray(np.asarray(emb_in), dtype=bf)
    w_ihT = dr_layout((np.asarray(W_ih).T.astype(np.float32) * WS).astype(f8))
    w_hhT = dr_layout((np.asarray(W_hh).T.astype(np.float32) * WS).astype(f8))
    bias = (np.asarray(b_ih) + np.asarray(b_hh)).astype(np.float32)
    bias_row = np.ascontiguousarray((bias * GS).astype(bf).reshape(1, G))
    emb_tgt_rows = np.ascontiguousarray(np.asarray(emb_tgt)[tgt0], dtype=bf)
    w_out_tgt = np.ascontiguousarray(np.asarray(W_out)[tgt0], dtype=bf)

    W_outT_pad = np.zeros((H, VPAD), dtype=bf)
    W_outT_pad[:, :V] = np.asarray(W_out).T.astype(bf)
    bout_pad = np.full((VPAD,), -1e30, dtype=np.float32)
    bout_pad[:V] = np.asarray(b_out).astype(np.float32)

    in_maps = []
    for k in range(NCORES):
        esel_k = np.zeros((H, ES), dtype=f8)
        for e in range(ES):
            esel_k[ES * k + e, e] = 1.0
        in_maps.append(
            dict(
                lines_cp=lines_cp,
                emb_in_bf=emb_in_bf,
                w_ihT=w_ihT,
                w_hhT=w_hhT,
                bias_row=bias_row,
                emb_tgt_rows=emb_tgt_rows,
                esel=dr_layout(esel_k),
                w_outT=np.ascontiguousarray(
                    W_outT_pad[:, VS * k : VS * (k + 1)].reshape(KC, 128, VS)
                ),
                bout_row=np.ascontiguousarray(
                    bout_pad[VS * k : VS * (k + 1)].reshape(1, VS)
                ),
                w_out_tgt=w_out_tgt,
            )
        )
    host_ctx = dict(tgt0=tgt0, b_out=np.asarray(b_out).astype(np.float64))
    return in_maps, host_ctx


def postprocess(results, host_ctx):
    tgt0 = host_ctx["tgt0"]
    b_out = host_ctx["b_out"]
    total_sumexp = np.zeros(B, dtype=np.float64)
    for r in results:
        total_sumexp += r["out_sumexp"][:, 0].astype(np.float64)
    ztgt = results[0]["out_ztgt"][:, 0].astype(np.float64) + b_out[tgt0]
    loss = float(np.mean(np.log(total_sumexp) - ztgt))
    at = np.zeros((H, H), dtype=np.float32)
    for k, r in enumerate(results):
        at[:, ES * k : ES * (k + 1)] = r["out_at"].reshape(H, ES)
    return np.float32(loss), at


# ---------------- cached runner ----------------

_RUNNER = None


class _Runner:
    """Builds the Bass program once and keeps a reusable sharded-jit callable
    (mirrors concourse.bass2jax.run_bass_via_pjrt, but reusable so repeat
    calls don't recompile and inputs can stay device-resident for timing)."""

    def __init__(self, Tn=256, reps=1):
        import jax
        from jax.sharding import Mesh, PartitionSpec
        from jax.experimental.shard_map import shard_map
        from concourse import bass2jax

        bass2jax.install_neuronx_cc_hook()
        self.jax = jax
        self.PartitionSpec = PartitionSpec
        nc = build_nc(Tn=Tn, reps=reps)
        split_multi_waits(nc)
        self.nc = nc

        partition_name = (
            nc.partition_id_tensor.name if nc.partition_id_tensor else None
        )
        in_names, out_names, out_avals, zero_outs = [], [], [], []
        for alloc in nc.m.functions[0].allocations:
            if not isinstance(alloc, mybir.MemoryLocationSet):
                continue
            name = alloc.memorylocations[0].name
            if alloc.kind == "ExternalInput":
                if name != partition_name:
                    in_names.append(name)
            elif alloc.kind == "ExternalOutput":
                shape = tuple(alloc.tensor_shape)
                dt = mybir.dt.np(alloc.dtype)
                out_names.append(name)
                out_avals.append(jax.core.ShapedArray(shape, dt))
                zero_outs.append(np.zeros(shape, dt))
        self.in_names = list(in_names)
        self.out_names = out_names
        self.zero_outs = zero_outs
        n_params = len(in_names)
        n_outs = len(out_names)
        all_in_names = in_names + out_names
        if partition_name is not None:
            all_in_names = all_in_names + [partition_name]

        def _body(*args):
            operands = list(args)
            if partition_name is not None:
                operands.append(bass2jax.partition_id_tensor())
            outs = bass2jax._bass_exec_p.bind(
                *operands,
                out_avals=tuple(out_avals),
                in_names=tuple(all_in_names),
                out_names=tuple(out_names),
                lowering_input_output_aliases=(),
                sim_require_finite=True,
                sim_require_nnan=True,
                nc=nc,
            )
            return tuple(outs)

        devices = jax.devices()[:NCORES]
        self.mesh = Mesh(np.asarray(devices), ("core",))
        in_specs = (PartitionSpec("core"),) * (n_params + n_outs)
        out_specs = (PartitionSpec("core"),) * n_outs
        self.donate = tuple(range(n_params, n_params + n_outs))
        self.sharded = jax.jit(
            shard_map(
                _body,
                mesh=self.mesh,
                in_specs=in_specs,
                out_specs=out_specs,
                check_rep=False,
            ),
            donate_argnums=self.donate,
            keep_unused=True,
        )

    def concat_inputs(self, in_maps):
        return [
            np.concatenate([np.asarray(in_maps[c][n]) for c in range(NCORES)], axis=0)
            for n in self.in_names
        ]

    def _zeros(self):
        return [
            np.zeros((NCORES * z.shape[0], *z.shape[1:]), z.dtype)
            for z in self.zero_outs
        ]

    def execute(self, concat_in):
        out_arrs = self.sharded(*concat_in, *self._zeros())
        results = [{} for _ in range(NCORES)]
        for i, name in enumerate(self.out_names):
            full = np.asarray(out_arrs[i])
            per = full.reshape((NCORES, full.shape[0] // NCORES) + full.shape[1:])
            for c in range(NCORES):
                results[c][name] = per[c]
        return results

    def run(self, in_maps):
        return self.execute(self.concat_inputs(in_maps))


def _get_runner():
    global _RUNNER
    if _RUNNER is None:
        _RUNNER = _Runner(Tn=256)
    return _RUNNER


def kernel(**inputs):
    r = _get_runner()
    in_maps, host_ctx = prep_inputs(**inputs)
    results = r.run(in_maps)
    return postprocess(results, host_ctx)


def _time_runner(r, dev_in, iters=7):
    import time as _time

    jax = r.jax
    jax.block_until_ready(r.sharded(*dev_in, *r._zeros()))
    times = []
    for _ in range(iters):
        t0 = _time.perf_counter()
        jax.block_until_ready(r.sharded(*dev_in, *r._zeros()))
        times.append(_time.perf_counter() - t0)
    times.sort()
    return times[len(times) // 2]


def time_kernel_ns(inputs, reps_hi=8):
    """Estimate device time per kernel execution by the slope between an
    R=1 and an R=reps_hi in-program repetition build (removes the large
    fixed axon dispatch overhead)."""
    from jax.sharding import NamedSharding

    r1 = _get_runner()
    in_maps, _ = prep_inputs(**inputs)
    concat_in = r1.concat_inputs(in_maps)
    jax = r1.jax
    sh = NamedSharding(r1.mesh, r1.PartitionSpec("core"))
    dev_in = [jax.device_put(a, sh) for a in concat_in]
    t1 = _time_runner(r1, dev_in)
    rh = _Runner(Tn=256, reps=reps_hi)
    th = _time_runner(rh, dev_in)
    return int((th - t1) / (reps_hi - 1) * 1e9)


# revision 15
# speedup vs baseline: 108.5754x; 3.4680x over previous
"""nn_Encoder_Decoder kernel for 8 trn2 NeuronCores (Bass/Tile, SPMD).

Contract: kernel(**inputs) takes the FULL unsharded inputs of
reference.setup_inputs() and returns the FULL output (loss scalar, at[512,512]).

Sharding strategy (per spec hint, adapted):
  - The 256-step LSTM scan is replicated on all 8 cores: it is latency-bound
    (per-step time is independent of batch), and the batch-contracted
    attention einsum needs full-batch hs on every core anyway.
  - Per-core shards: 1/8 of the attention output columns (one-hot-selected
    e-slice of hs kept during the scan) and 1/8 of the vocab for the
    output-projection softmax denominator (tensor parallel over [V,H]).
  - The x-side gate projections (x_t @ W_ih.T + b) are interleaved with the
    scan and accumulated straight into the gate PSUM (two timesteps share a
    [128, 2048] psum pair; the odd step lands on partitions 64-127 via
    tile_position col placement), so no staging and no separate add pass.
  - Recurrent/x projections run in fp8-e4m3 DoubleRow (h pre-scaled by 64,
    weights by 16; the 1/1024 washout rides the activation's free affine).
    fp32 c-state and all softmax/attention statistics.

Host glue is O(B + H^2): sum 8 partial sumexp vectors -> log -> loss;
concatenate the 8 at column-slices.
"""

import os
import sys

for _p in ("/opt/trn_rl_repo", "/root/.axon_site/_ro/trn_rl_repo"):
    if os.path.isdir(_p) and _p not in sys.path:
        sys.path.append(_p)

import numpy as np
import ml_dtypes

import concourse.bass as bass
import concourse.mybir as mybir
from concourse.tile import TileContext
from concourse.masks import make_identity

BF16 = mybir.dt.bfloat16
F32 = mybir.dt.float32
I32 = mybir.dt.int32
F8 = mybir.dt.float8e4
AF = mybir.ActivationFunctionType
ALU = mybir.AluOpType
DR = mybir.MatmulPerfMode.DoubleRow

V = 50257
H = 512
B = 64
G = 4 * H  # 2048
NCORES = 8
ES = H // NCORES  # 64 attention-output columns per core
VS = 13 * 512  # 6656 vocab columns per core (padded; 8*6656 >= V)
VPAD = NCORES * VS
KC = H // 128  # 4 bf16 contraction chunks of the H dim
KK = 2  # fp8 DoubleRow contraction chunks (2 x 256)
HS = 64.0  # fp8 scale on h / x
WS = 16.0  # fp8 scale on W_ih / W_hh
GS = HS * WS  # resulting gate pre-activation scale in PSUM


def split_multi_waits(nc):
    """Split instructions with >1 sync wait into single-wait NoOps.

    The walrus build in this container caps sync waits at 1 per instruction;
    TileContext emits instructions with several. Waits run before an
    instruction's action and engines execute their stream in order, so
    hoisting all-but-one wait onto preceding NoOps is semantically identical.
    """
    n_split = 0
    for f in nc.m.functions:
        for bb in f.blocks:
            insts = bb.instructions
            new_list = []
            for inst in insts:
                si = getattr(inst, "sync_info", None)
                if si is not None and si.on_wait is not None and len(si.on_wait) > 1:
                    waits = list(si.on_wait)
                    for j, w in enumerate(waits[:-1]):
                        n_split += 1
                        new_list.append(
                            mybir.InstNoOp(
                                name=f"{inst.name}-wsplit{j}",
                                engine=inst.engine,
                                debug=getattr(inst, "debug", None),
                                ins=[],
                                outs=[],
                                sync_info=mybir.SyncInfo(on_wait=[w], on_update=[]),
                            )
                        )
                    si.on_wait = [waits[-1]]
                new_list.append(inst)
            del insts[:]
            insts.extend(new_list)
    return n_split


def build_nc(Tn=256, reps=1):
    assert Tn % 2 == 0
    CH = Tn * B // 128  # token chunks of 128 (= 2 timesteps each)
    nc = bass.Bass("TRN2", target_bir_lowering=False, debug=False, num_devices=NCORES)

    lines_cp = nc.dram_tensor("lines_cp", [128, CH], I32, kind="ExternalInput")
    emb_in_bf = nc.dram_tensor("emb_in_bf", [V, H], BF16, kind="ExternalInput")
    w_ihT = nc.dram_tensor("w_ihT", [KK, 128, 2, G], F8, kind="ExternalInput")
    w_hhT = nc.dram_tensor("w_hhT", [KK, 128, 2, G], F8, kind="ExternalInput")
    bias_row = nc.dram_tensor("bias_row", [1, G], BF16, kind="ExternalInput")
    emb_tgt_rows = nc.dram_tensor("emb_tgt_rows", [B, H], BF16, kind="ExternalInput")
    esel = nc.dram_tensor("esel", [KK, 128, 2, ES], F8, kind="ExternalInput")
    w_outT = nc.dram_tensor("w_outT", [KC, 128, VS], BF16, kind="ExternalInput")
    bout_row = nc.dram_tensor("bout_row", [1, VS], F32, kind="ExternalInput")
    w_out_tgt = nc.dram_tensor("w_out_tgt", [B, H], BF16, kind="ExternalInput")

    out_at = nc.dram_tensor("out_at", [KC, 128, ES], F32, kind="ExternalOutput")
    out_sumexp = nc.dram_tensor("out_sumexp", [B, 1], F32, kind="ExternalOutput")
    out_ztgt = nc.dram_tensor("out_ztgt", [B, 1], F32, kind="ExternalOutput")

    with TileContext(nc) as tc:
        with tc.tile_pool(name="singles", bufs=1) as sing:
            ident = sing.tile([128, 128], BF16, tag="ident")
            make_identity(nc, ident[:])
            w_ihT_s = sing.tile([128, KK, 2, G], F8, tag="wih")
            w_hhT_s = sing.tile([128, KK, 2, G], F8, tag="whh")
            for kk in range(KK):
                nc.sync.dma_start(w_ihT_s[:, kk, :, :], w_ihT[kk])
                nc.sync.dma_start(w_hhT_s[:, kk, :, :], w_hhT[kk])
            bias_s = sing.tile([1, G], BF16, tag="bias")
            nc.sync.dma_start(bias_s[:], bias_row[:, :])
            esel_s = sing.tile([128, KK, 2, ES], F8, tag="esel")
            for kk in range(KK):
                nc.sync.dma_start(esel_s[:, kk, :, :], esel[kk])
            lines_s = sing.tile([128, CH], I32, tag="lines")
            nc.sync.dma_start(lines_s[:], lines_cp[:, :])
            hs_store = sing.tile([B, Tn * ES], BF16, tag="hs_store")
            emb_tgt_s = sing.tile([B, H], BF16, tag="embtgt")
            nc.sync.dma_start(emb_tgt_s[:], emb_tgt_rows[:, :])
            w_out_tgt_s = sing.tile([B, H], BF16, tag="wouttgt")
            nc.sync.dma_start(w_out_tgt_s[:], w_out_tgt[:, :])
            ones_s = sing.tile([1, 128], BF16, tag="ones")
            nc.vector.memset(ones_s[:], 1.0)
            hs_keep = sing.tile([B, H], BF16, tag="hs_keep")
            hsT_keep = sing.tile([128, KC, B], BF16, tag="hsT_keep")
            ht_keep = sing.tile([B, H], BF16, tag="ht_keep")

            for _rep in range(reps):
                # ---- scan with interleaved x-side projections ----
                with (
                    tc.tile_pool(name="sc_state", bufs=2) as scs,
                    tc.tile_pool(name="sc_work", bufs=2) as scw,
                    tc.tile_pool(name="pb_sb", bufs=3) as pb,
                    tc.tile_pool(name="pb_dram", bufs=4, space="DRAM") as pbd,
                ):
                    with (
                        tc.tile_pool(name="sc_ps_big", bufs=3, space="PSUM") as bigp,
                        tc.tile_pool(name="sc_ps_aux", bufs=2, space="PSUM") as auxp,
                    ):
                        c_cur = scs.tile([B, H], F32, tag="c_state")
                        nc.vector.memset(c_cur[:], 0.0)
                        hT_cur = scs.tile([128, KK, 2, B], F8, tag="hT")
                        nc.vector.memset(hT_cur[:], 0.0)
                        last_ps_hT = None

                        def x_side(xT_lhsT, col0):
                            """Emit x@W_ih.T + b for ONE step (batch cols
                            [col0, col0+64) of the chunk's xT) into two fresh
                            [64,1024] psum tiles (starts the accum group).
                            xT_lhsT: fp8 [128, KK, 2, >=col0+64] (x HS)."""
                            bigA = bigp.tile([B, 1024], F32, tag="big")
                            bigB = bigp.tile([B, 1024], F32, tag="big")
                            for bt, ns in ((bigA, (0, 1)), (bigB, (2, 3))):
                                for j, n in enumerate(ns):
                                    sl = bt[:, 512 * j : 512 * (j + 1)]
                                    for kk in range(KK):
                                        nc.tensor.matmul(
                                            sl,
                                            xT_lhsT[:, kk, :, col0 : col0 + B],
                                            w_ihT_s[:, kk, :, 512 * n : 512 * (n + 1)],
                                            start=(kk == 0),
                                            stop=False,
                                            perf_mode=DR,
                                        )
                                    nc.tensor.matmul(
                                        sl,
                                        ones_s[:, 0:B],
                                        bias_s[:, 512 * n : 512 * (n + 1)],
                                        start=False,
                                        stop=False,
                                    )
                            return bigA, bigB

                        def lstm_step(bigA, bigB, t_store):
                            """One LSTM cell step; gates in psum rows [0, 64)
                            of bigA (i,f) / bigB (g,o)."""
                            nonlocal c_cur, hT_cur, last_ps_hT
                            for bt, ns in ((bigA, (0, 1)), (bigB, (2, 3))):
                                for j, n in enumerate(ns):
                                    sl = bt[:, 512 * j : 512 * (j + 1)]
                                    for kk in range(KK):
                                        nc.tensor.matmul(
                                            sl,
                                            hT_cur[:, kk, :, :],
                                            w_hhT_s[:, kk, :, 512 * n : 512 * (n + 1)],
                                            start=False,
                                            stop=(kk == KK - 1),
                                            perf_mode=DR,
                                        )
                            gates = scw.tile([B, G], BF16, tag="gates")
                            nc.scalar.activation(
                                gates[:, 0:1024], bigA[:, :], AF.Sigmoid,
                                scale=1.0 / GS,
                            )
                            nc.scalar.activation(
                                gates[:, 1024:1536], bigB[:, 0:512], AF.Tanh,
                                scale=1.0 / GS,
                            )
                            nc.scalar.activation(
                                gates[:, 1536:2048], bigB[:, 512:1024], AF.Sigmoid,
                                scale=1.0 / GS,
                            )
                            i_ = gates[:, 0:512]
                            f_ = gates[:, 512:1024]
                            g_ = gates[:, 1024:1536]
                            o_ = gates[:, 1536:2048]
                            t1 = scw.tile([B, H], F32, tag="t1")
                            nc.vector.tensor_mul(t1[:], i_, g_)
                            c_new = scs.tile([B, H], F32, tag="c_state")
                            nc.gpsimd.tensor_mul(c_new[:], f_, c_cur[:])
                            nc.vector.tensor_add(c_new[:], c_new[:], t1[:])
                            tc_t = scw.tile([B, H], BF16, tag="tanh_c")
                            nc.scalar.activation(tc_t[:], c_new[:], AF.Tanh)
                            h = scw.tile([B, H], BF16, tag="h")
                            nc.vector.tensor_mul(h[:], o_, tc_t[:])
                            ps_hT = auxp.tile(
                                [128, KC, 128], BF16, tag="aux", name="ps_hT"
                            )
                            last_ps_hT = ps_hT
                            for k in range(KC):
                                nc.tensor.transpose(
                                    ps_hT[:, k, 0:B],
                                    h[:, 128 * k : 128 * (k + 1)],
                                    ident[0:B, 0:B],
                                )
                            hT_new = scs.tile([128, KK, 2, B], F8, tag="hT")
                            nc.vector.tensor_scalar_mul(hT_new[:], ps_hT[:, :, 0:B], HS)
                            if t_store >= 0:
                                ps_sl = auxp.tile([B, ES], F32, tag="aux", name="ps_sl")
                                for kk in range(KK):
                                    nc.tensor.matmul(
                                        ps_sl[:],
                                        hT_new[:, kk, :, :],
                                        esel_s[:, kk, :, :],
                                        start=(kk == 0),
                                        stop=(kk == KK - 1),
                                        perf_mode=DR,
                                    )
                                nc.vector.tensor_scalar_mul(
                                    hs_store[:, ES * t_store : ES * (t_store + 1)],
                                    ps_sl[:],
                                    1.0 / HS,
                                )
                            c_cur = c_new
                            hT_cur = hT_new
                            return h

                        def fetch_chunk(c):
                            xg = pb.tile([128, H], BF16, tag="xg", name=f"xg{c}")
                            nc.gpsimd.indirect_dma_start(
                                out=xg[:],
                                out_offset=None,
                                in_=emb_in_bf[:],
                                in_offset=bass.IndirectOffsetOnAxis(
                                    ap=lines_s[:, c : c + 1], axis=0
                                ),
                            )
                            # bounce via DRAM: HW xbar transpose is DRAM->SBUF
                            xg_d = pbd.tile([128, H], BF16, tag="xgd", name=f"xgd{c}")
                            nc.sync.dma_start(xg_d[:], xg[:])
                            xgT = pb.tile([128, KC, 128], BF16, tag="xgT", name=f"xgT{c}")
                            for k in range(KC):
                                nc.sync.dma_start(
                                    xgT[:, k, :],
                                    xg_d[:, 128 * k : 128 * (k + 1)],
                                    transpose=True,
                                )
                            xT = pb.tile([128, KK, 2, 128], F8, tag="xT", name=f"xT{c}")
                            nc.gpsimd.tensor_copy(xT[:], xgT[:])
                            return xT

                        LOOKAHEAD = 3
                        xT_q = [fetch_chunk(c) for c in range(min(LOOKAHEAD, CH))]
                        h_nat = None
                        for c in range(CH):
                            xT = xT_q[c]
                            if c + LOOKAHEAD < CH:
                                xT_q.append(fetch_chunk(c + LOOKAHEAD))
                            for half in range(2):
                                bigA, bigB = x_side(xT, B * half)
                                h_nat = lstm_step(bigA, bigB, 2 * c + half)

                        nc.vector.tensor_copy(hs_keep[:], h_nat[:])
                        nc.vector.tensor_copy(hsT_keep[:], last_ps_hT[:, :, 0:B])

                        # decoder step (x = emb_tgt rows, M=64)
                        ps_xd = auxp.tile([128, KC, 128], BF16, tag="aux", name="ps_xd")
                        for k in range(KC):
                            nc.tensor.transpose(
                                ps_xd[:, k, 0:B],
                                emb_tgt_s[:, 128 * k : 128 * (k + 1)],
                                ident[0:B, 0:B],
                            )
                        xdT = scw.tile([128, KK, 2, B], F8, tag="xdT", bufs=1)
                        nc.vector.tensor_copy(xdT[:], ps_xd[:, :, 0:B])
                        bigA, bigB = x_side(xdT, 0)
                        ht_nat = lstm_step(bigA, bigB, -1)
                        nc.vector.tensor_copy(ht_keep[:], ht_nat[:])

                        # z_at_target = <hs, W_out[tgt]> rowwise
                        ztg_scr = scw.tile([B, H], F32, tag="ztg_scr", bufs=1)
                        ztg = scw.tile([B, 1], F32, tag="ztg", bufs=1)
                        nc.vector.tensor_mul(ztg_scr[:], hs_keep[:], w_out_tgt_s[:])
                        nc.vector.tensor_reduce(
                            out=ztg[:], in_=ztg_scr[:], axis=mybir.AxisListType.X,
                            op=ALU.add,
                        )
                        nc.sync.dma_start(out_ztgt[:, :], ztg[:])

                    # ---- attention slice ----
                    TG = min(32, Tn)
                    n_tg = Tn // TG
                    MMW = min(512, TG * ES)
                    with (
                        tc.tile_pool(name="at_ps", bufs=2, space="PSUM") as atps,
                        tc.tile_pool(name="at_sb", bufs=2) as atsb,
                        tc.tile_pool(name="at_acc", bufs=1) as atacc,
                    ):
                        acc = atacc.tile([128, KC, ES], F32, tag="acc")
                        exp0 = atacc.tile([128, KC, ES], F32, tag="exp0")
                        for m in range(KC):
                            for g in range(n_tg):
                                ps_s = atps.tile([128, TG * ES], F32, tag="ps_s")
                                for q in range(TG * ES // MMW):
                                    nc.tensor.matmul(
                                        ps_s[:, MMW * q : MMW * (q + 1)],
                                        ht_keep[:, 128 * m : 128 * (m + 1)],
                                        hs_store[
                                            :,
                                            TG * ES * g
                                            + MMW * q : TG * ES * g
                                            + MMW * (q + 1),
                                        ],
                                        start=True,
                                        stop=True,
                                    )
                                ex = atsb.tile([128, TG * ES], F32, tag="ex")
                                nc.scalar.activation(ex[:], ps_s[:], AF.Exp)
                                if g == 0:
                                    nc.vector.tensor_copy(exp0[:, m, :], ex[:, 0:ES])
                                w = TG * ES // 2
                                while w >= ES:
                                    nc.vector.tensor_add(
                                        ex[:, 0:w], ex[:, 0:w], ex[:, w : 2 * w]
                                    )
                                    w //= 2
                                if g == 0:
                                    nc.vector.tensor_copy(acc[:, m, :], ex[:, 0:ES])
                                else:
                                    nc.vector.tensor_add(
                                        acc[:, m, :], acc[:, m, :], ex[:, 0:ES]
                                    )
                        rec = atsb.tile([128, KC, ES], F32, tag="rec")
                        nc.vector.reciprocal(rec[:], acc[:])
                        at_t = atsb.tile([128, KC, ES], F32, tag="at")
                        nc.vector.tensor_mul(at_t[:], exp0[:], rec[:])
                        for m in range(KC):
                            nc.sync.dma_start(out_at[m], at_t[:, m, :])

                    # ---- vocab-shard sumexp of logits ----
                    with (
                        tc.tile_pool(name="lg_ps", bufs=4, space="PSUM") as lgps,
                        tc.tile_pool(name="lg_sb", bufs=2) as lgsb,
                        tc.tile_pool(name="lg_acc", bufs=1) as lgacc,
                    ):
                        nchunk = VS // 512
                        w_outT_s = lgacc.tile([128, KC, VS], BF16, tag="wout")
                        for k in range(KC):
                            nc.sync.dma_start(w_outT_s[:, k, :], w_outT[k])
                        bout_s = lgacc.tile([1, VS], F32, tag="bout")
                        nc.sync.dma_start(bout_s[:], bout_row[:, :])
                        sums = lgacc.tile([B, nchunk], F32, tag="sums")
                        for cix in range(nchunk):
                            ps_l = lgps.tile([B, 512], F32, tag="ps_l")
                            for k in range(KC):
                                nc.tensor.matmul(
                                    ps_l[:],
                                    hsT_keep[:, k, :],
                                    w_outT_s[:, k, 512 * cix : 512 * (cix + 1)],
                                    start=(k == 0),
                                    stop=False,
                                )
                            bz = lgsb.tile([1, 512], BF16, tag="bz")
                            nc.vector.tensor_copy(
                                bz[:], bout_s[:, 512 * cix : 512 * (cix + 1)]
                            )
                            nc.tensor.matmul(
                                ps_l[:], ones_s[:, 0:B], bz[:], start=False, stop=True
                            )
                            scr = lgsb.tile([B, 512], BF16, tag="scr")
                            nc.scalar.activation(
                                scr[:], ps_l[:], AF.Exp,
                                accum_out=sums[:, cix : cix + 1],
                            )
                        tot = lgacc.tile([B, 1], F32, tag="tot")
                        nc.vector.tensor_reduce(
                            out=tot[:], in_=sums[:], axis=mybir.AxisListType.X,
                            op=ALU.add,
                        )
                        nc.sync.dma_start(out_sumexp[:, :], tot[:])

    return nc


def prep_inputs(input_lines, target_lines, emb_in, emb_tgt, W_ih, W_hh, b_ih, b_hh,
                W_out, b_out, Tn=256):
    bf = ml_dtypes.bfloat16
    f8 = ml_dtypes.float8_e4m3fn
    input_lines = np.asarray(input_lines)[:Tn]
    tgt0 = np.asarray(target_lines)[0].astype(np.int64)

    CH = Tn * B // 128
    lines_flat = np.asarray(input_lines).reshape(-1).astype(np.int32)
    lines_cp = np.ascontiguousarray(lines_flat.reshape(CH, 128).T)

    def dr_layout(WT):
        # [H, cols] -> [KK, 128, 2, cols] with j = 256*kk + 128*ko + ki
        cols = WT.shape[1]
        return np.ascontiguousarray(WT.reshape(KK, 2, 128, cols).transpose(0, 2, 1, 3))

    emb_in_bf = np.ascontiguousarray(np.asarray(emb_in) * HS, dtype=bf)
    w_ihT = dr_layout((np.asarray(W_ih).T.astype(np.float32) * WS).astype(f8))
    w_hhT = dr_layout((np.asarray(W_hh).T.astype(np.float32) * WS).astype(f8))
    bias = (np.asarray(b_ih) + np.asarray(b_hh)).astype(np.float32)
    bias_row = np.ascontiguousarray((bias * GS).astype(bf).reshape(1, G))
    emb_tgt_rows = np.ascontiguousarray(np.asarray(emb_tgt)[tgt0] * HS, dtype=bf)
    w_out_tgt = np.ascontiguousarray(np.asarray(W_out)[tgt0], dtype=bf)

    W_outT_pad = np.zeros((H, VPAD), dtype=bf)
    W_outT_pad[:, :V] = np.asarray(W_out).T.astype(bf)
    bout_pad = np.full((VPAD,), -1e30, dtype=np.float32)
    bout_pad[:V] = np.asarray(b_out).astype(np.float32)

    in_maps = []
    for k in range(NCORES):
        esel_k = np.zeros((H, ES), dtype=f8)
        for e in range(ES):
            esel_k[ES * k + e, e] = 1.0
        in_maps.append(
            dict(
                lines_cp=lines_cp,
                emb_in_bf=emb_in_bf,
                w_ihT=w_ihT,
                w_hhT=w_hhT,
                bias_row=bias_row,
                emb_tgt_rows=emb_tgt_rows,
                esel=dr_layout(esel_k),
                w_outT=np.ascontiguousarray(
                    W_outT_pad[:, VS * k : VS * (k + 1)].reshape(KC, 128, VS)
                ),
                bout_row=np.ascontiguousarray(
                    bout_pad[VS * k : VS * (k + 1)].reshape(1, VS)
                ),
                w_out_tgt=w_out_tgt,
            )
        )
    host_ctx = dict(tgt0=tgt0, b_out=np.asarray(b_out).astype(np.float64))
    return in_maps, host_ctx


def postprocess(results, host_ctx):
    tgt0 = host_ctx["tgt0"]
    b_out = host_ctx["b_out"]
    total_sumexp = np.zeros(B, dtype=np.float64)
    for r in results:
        total_sumexp += r["out_sumexp"][:, 0].astype(np.float64)
    ztgt = results[0]["out_ztgt"][:, 0].astype(np.float64) + b_out[tgt0]
    loss = float(np.mean(np.log(total_sumexp) - ztgt))
    at = np.zeros((H, H), dtype=np.float32)
    for k, r in enumerate(results):
        at[:, ES * k : ES * (k + 1)] = r["out_at"].reshape(H, ES)
    return np.float32(loss), at


# ---------------- cached runner ----------------

_RUNNER = None


class _Runner:
    """Builds the Bass program once and keeps a reusable sharded-jit callable
    (mirrors concourse.bass2jax.run_bass_via_pjrt, but reusable so repeat
    calls don't recompile and inputs can stay device-resident for timing)."""

    def __init__(self, Tn=256, reps=1):
        import jax
        from jax.sharding import Mesh, PartitionSpec
        from jax.experimental.shard_map import shard_map
        from concourse import bass2jax

        bass2jax.install_neuronx_cc_hook()
        self.jax = jax
        self.PartitionSpec = PartitionSpec
        nc = build_nc(Tn=Tn, reps=reps)
        split_multi_waits(nc)
        self.nc = nc

        partition_name = (
            nc.partition_id_tensor.name if nc.partition_id_tensor else None
        )
        in_names, out_names, out_avals, zero_outs = [], [], [], []
        for alloc in nc.m.functions[0].allocations:
            if not isinstance(alloc, mybir.MemoryLocationSet):
                continue
            name = alloc.memorylocations[0].name
            if alloc.kind == "ExternalInput":
                if name != partition_name:
                    in_names.append(name)
            elif alloc.kind == "ExternalOutput":
                shape = tuple(alloc.tensor_shape)
                dt = mybir.dt.np(alloc.dtype)
                out_names.append(name)
                out_avals.append(jax.core.ShapedArray(shape, dt))
                zero_outs.append(np.zeros(shape, dt))
        self.in_names = list(in_names)
        self.out_names = out_names
        self.zero_outs = zero_outs
        n_params = len(in_names)
        n_outs = len(out_names)
        all_in_names = in_names + out_names
        if partition_name is not None:
            all_in_names = all_in_names + [partition_name]

        def _body(*args):
            operands = list(args)
            if partition_name is not None:
                operands.append(bass2jax.partition_id_tensor())
            outs = bass2jax._bass_exec_p.bind(
                *operands,
                out_avals=tuple(out_avals),
                in_names=tuple(all_in_names),
                out_names=tuple(out_names),
                lowering_input_output_aliases=(),
                sim_require_finite=True,
                sim_require_nnan=True,
                nc=nc,
            )
            return tuple(outs)

        devices = jax.devices()[:NCORES]
        self.mesh = Mesh(np.asarray(devices), ("core",))
        in_specs = (PartitionSpec("core"),) * (n_params + n_outs)
        out_specs = (PartitionSpec("core"),) * n_outs
        self.donate = tuple(range(n_params, n_params + n_outs))
        self.sharded = jax.jit(
            shard_map(
                _body,
                mesh=self.mesh,
                in_specs=in_specs,
                out_specs=out_specs,
                check_rep=False,
            ),
            donate_argnums=self.donate,
            keep_unused=True,
        )

    def concat_inputs(self, in_maps):
        return [
            np.concatenate([np.asarray(in_maps[c][n]) for c in range(NCORES)], axis=0)
            for n in self.in_names
        ]

    def _zeros(self):
        return [
            np.zeros((NCORES * z.shape[0], *z.shape[1:]), z.dtype)
            for z in self.zero_outs
        ]

    def execute(self, concat_in):
        out_arrs = self.sharded(*concat_in, *self._zeros())
        results = [{} for _ in range(NCORES)]
        for i, name in enumerate(self.out_names):
            full = np.asarray(out_arrs[i])
            per = full.reshape((NCORES, full.shape[0] // NCORES) + full.shape[1:])
            for c in range(NCORES):
                results[c][name] = per[c]
        return results

    def run(self, in_maps):
        return self.execute(self.concat_inputs(in_maps))


def _get_runner():
    global _RUNNER
    if _RUNNER is None:
        _RUNNER = _Runner(Tn=256)
    return _RUNNER


def kernel(**inputs):
    r = _get_runner()
    in_maps, host_ctx = prep_inputs(**inputs)
    results = r.run(in_maps)
    return postprocess(results, host_ctx)


def _time_runner(r, dev_in, iters=7):
    import time as _time

    jax = r.jax
    jax.block_until_ready(r.sharded(*dev_in, *r._zeros()))
    times = []
    for _ in range(iters):
        t0 = _time.perf_counter()
        jax.block_until_ready(r.sharded(*dev_in, *r._zeros()))
        times.append(_time.perf_counter() - t0)
    times.sort()
    return times[len(times) // 2]


def time_kernel_ns(inputs, reps_hi=8):
    """Estimate device time per kernel execution by the slope between an
    R=1 and an R=reps_hi in-program repetition build (removes the large
    fixed axon dispatch overhead)."""
    from jax.sharding import NamedSharding

    r1 = _get_runner()
    in_maps, _ = prep_inputs(**inputs)
    concat_in = r1.concat_inputs(in_maps)
    jax = r1.jax
    sh = NamedSharding(r1.mesh, r1.PartitionSpec("core"))
    dev_in = [jax.device_put(a, sh) for a in concat_in]
    t1 = _time_runner(r1, dev_in)
    rh = _Runner(Tn=256, reps=reps_hi)
    th = _time_runner(rh, dev_in)
    return int((th - t1) / (reps_hi - 1) * 1e9)
